# revision 8
# baseline (speedup 1.0000x reference)
"""Trainium2 Bass kernel for the PNODE+decoder reference (RK4 latent ODE,
linear trajectory interpolation, Fourier-feature decoder, hard-constraint PINN
output), data-parallel over 8 NeuronCores.

Layout (feature-major, batch on the free dim):
  per core B_CORE=4096 batch elements, 4 groups x 1024 columns.
  Z tile [128, 1024] per group:  rows 0-9 k1s, 32-41 k2s, 64-73 k3s,
  96-105 state a, 106 mu (k's are h-scaled, bias-free; all bias terms are
  folded into a per-(step,eval) ACT bias table and a decoder correction row).
  RK4 state combinations are folded into padded matmul weights, so each eval
  is: mm1(K=128) -> tanh -> mm2 -> tanh -> mm3(M=10) -> copy k back into Z.
"""

import numpy as np

B = 32768
NCORES = 8
B_CORE = B // NCORES          # 4096
NG = 4                        # groups per core
GW = B_CORE // NG             # 1024 columns per group
STEPS = 101
NSTEP = STEPS - 1             # 100 RK4 steps
T_END = 1.0
H = T_END / NSTEP
LATENT = 10
HIDDEN = 128
N_FREQS = 16
MAX_FREQ = 10.0

_PROG_CACHE = {}


def _split_multiwaits(nc, mybir):
    """This walrus accepts at most 1 sync-wait per instruction (2 for
    EventSemaphore). Tile's exit drain can carry more; hoist extras onto
    standalone NoOps inserted right before the offending instruction."""
    n = 0
    for f in nc.m.functions:
        for b in f.blocks:
            out = []
            for inst in b.instructions:
                si = inst.sync_info
                waits = list(si.on_wait) if si and si.on_wait else []
                cap = 2 if isinstance(inst, mybir.InstEventSemaphore) else 1
                if len(waits) > cap:
                    extra, keep = waits[:-cap], waits[-cap:]
                    for w in extra:
                        n += 1
                        out.append(mybir.InstNoOp(
                            name=f"{inst.name}-ws{n}", engine=inst.engine,
                            sync_info=mybir.SyncInfo(on_wait=[w], on_update=[])))
                    inst.sync_info = mybir.SyncInfo(
                        on_wait=keep, on_update=list(si.on_update or []))
                out.append(inst)
            b.instructions[:] = out
    return n


def _row_ap(bass, ap, nrows=1):
    """View a 1-D DRAM AP as [nrows, N] via partition step 0 (broadcast)."""
    return bass.AP(tensor=ap.tensor, offset=ap.offset,
                   ap=[[0, nrows]] + [list(d) for d in ap.ap])


def _build(nstep):
    import concourse.bass as bass
    import concourse.tile as tile
    import concourse.mybir as mybir

    f32 = mybir.dt.float32
    AF = mybir.ActivationFunctionType
    OP = mybir.AluOpType

    nc = bass.Bass('TRN2', target_bir_lowering=False, debug=False)

    def din(name, shape):
        return nc.dram_tensor(name, shape, f32, kind='ExternalInput')

    x_d = din('x', [B_CORE])
    t_d = din('t', [B_CORE])
    mu_d = din('mu', [B_CORE])
    V_d = din('V', [nstep + 1, B_CORE])
    w1e_d = din('w1e', [128, 512])
    w2_d = din('w2', [128, 128])
    w3h_d = din('w3h', [128, LATENT])
    w3h8_d = din('w3h8', [128, LATENT])
    selw_d = din('selw', [128, LATENT])
    btab_d = din('btab', [128, 4 * nstep])
    b2c_d = din('b2c', [128, 1])
    dbias_d = din('dbias', [128, 4])
    db4c_d = din('db4c', [1, 1])
    dw1_d = din('dw1', [128, 128])
    dw2_d = din('dw2', [128, 128])
    dw3_d = din('dw3', [128, 128])
    dw4_d = din('dw4', [128, 1])
    fpad_d = din('fpad', [128, 65])
    u_d = nc.dram_tensor('u', [B_CORE], f32, kind='ExternalOutput')

    with tile.TileContext(nc) as tc:
        with tc.tile_pool(name='consts', bufs=1) as cpool, \
             tc.tile_pool(name='state', bufs=1) as spool, \
             tc.tile_pool(name='vbuf', bufs=2) as vpool, \
             tc.tile_pool(name='hbuf', bufs=6) as hpool, \
             tc.tile_pool(name='tbuf', bufs=3) as tpool, \
             tc.tile_pool(name='pp', bufs=4, space='PSUM') as pp:

            def cload(d, shape):
                t = cpool.tile(shape, f32, tag=d.name, name=f'c_{d.name}')
                nc.sync.dma_start(out=t, in_=d.ap())
                return t

            w1e = cload(w1e_d, [128, 512])
            w2 = cload(w2_d, [128, 128])
            w3h = cload(w3h_d, [128, LATENT])
            w3h8 = cload(w3h8_d, [128, LATENT])
            selw = cload(selw_d, [128, LATENT])
            btab = cload(btab_d, [128, 4 * nstep])
            b2c = cload(b2c_d, [128, 1])
            dbias = cload(dbias_d, [128, 4])
            db4c = cload(db4c_d, [1, 1])
            dw1 = cload(dw1_d, [128, 128])
            dw2 = cload(dw2_d, [128, 128])
            dw3 = cload(dw3_d, [128, 128])
            dw4 = cload(dw4_d, [128, 1])
            fpad = cload(fpad_d, [128, 65])

            Z = []
            Zdec = []
            trow = []
            for g in range(NG):
                zg = spool.tile([128, GW], f32, tag=f'Z{g}', name=f'Z{g}')
                zd = spool.tile([128, GW], f32, tag=f'Zd{g}', name=f'Zd{g}')
                nc.vector.memset(zg, 0.0)
                nc.vector.memset(zd, 0.0)
                gs = slice(g * GW, (g + 1) * GW)
                nc.gpsimd.dma_start(out=zg[106:107, :],
                                    in_=_row_ap(bass, mu_d.ap()[gs]))
                nc.gpsimd.dma_start(out=zd[107:108, :],
                                    in_=_row_ap(bass, x_d.ap()[gs]))
                nc.gpsimd.dma_start(out=zd[108:109, :],
                                    in_=_row_ap(bass, t_d.ap()[gs]))
                rg = spool.tile([128, GW], f32, tag=f'R{g}', name=f'R{g}')
                nc.gpsimd.dma_start(out=rg[0:1, :],
                                    in_=_row_ap(bass, t_d.ap()[gs]))
                Z.append(zg)
                Zdec.append(zd)
                trow.append(rg[0:1, :])

            def interp(s):
                vt = vpool.tile([128, B_CORE], f32, tag='vt', name='vt')
                nc.gpsimd.dma_start(out=vt[96:106, :],
                                    in_=_row_ap(bass, V_d.ap()[s], LATENT))
                for g in range(NG):
                    gs = slice(g * GW, (g + 1) * GW)
                    tmp = tpool.tile([128, GW], f32, tag='tmp', name='tmp')
                    nc.vector.tensor_tensor(out=tmp[96:106, :],
                                            in0=Z[g][96:106, :],
                                            in1=vt[96:106, gs], op=OP.mult)
                    nc.vector.tensor_tensor(out=Zdec[g][96:106, :],
                                            in0=Zdec[g][96:106, :],
                                            in1=tmp[96:106, :], op=OP.add)

            for s in range(nstep):
                if s > 0:
                    interp(s)
                for i in range(4):
                    bias_ap = btab[:, 4 * s + i: 4 * s + i + 1]
                    for g in range(NG):
                        pre1 = pp.tile([128, GW], f32, tag='pp')
                        for c in range(GW // 512):
                            cs = slice(c * 512, (c + 1) * 512)
                            nc.tensor.matmul(pre1[:, cs],
                                             w1e[:, i * 128:(i + 1) * 128],
                                             Z[g][:, cs], start=True, stop=True)
                        h1 = hpool.tile([128, GW], f32, tag='h')
                        nc.scalar.activation(out=h1, in_=pre1, func=AF.Tanh,
                                             bias=bias_ap)
                        pre2 = pp.tile([128, GW], f32, tag='pp')
                        for c in range(GW // 512):
                            cs = slice(c * 512, (c + 1) * 512)
                            nc.tensor.matmul(pre2[:, cs], w2, h1[:, cs],
                                             start=True, stop=True)
                        h2 = hpool.tile([128, GW], f32, tag='h')
                        nc.scalar.activation(out=h2, in_=pre2, func=AF.Tanh,
                                             bias=b2c[:, 0:1])
                        if i < 3:
                            qb = 32 * i
                            kp = pp.tile([128, GW], f32, tag='pp')
                            for c in range(GW // 512):
                                cs = slice(c * 512, (c + 1) * 512)
                                nc.tensor.matmul(kp[qb:qb + 10, cs], w3h,
                                                 h2[:, cs], start=True,
                                                 stop=True,
                                                 tile_position=(0, qb))
                            nc.vector.tensor_copy(out=Z[g][qb:qb + 10, :],
                                                  in_=kp[qb:qb + 10, :])
                        else:
                            sp = pp.tile([128, GW], f32, tag='pp')
                            for c in range(GW // 512):
                                cs = slice(c * 512, (c + 1) * 512)
                                nc.tensor.matmul(sp[96:106, cs], selw,
                                                 Z[g][:, cs], start=True,
                                                 stop=False,
                                                 tile_position=(0, 96))
                                nc.tensor.matmul(sp[96:106, cs], w3h8,
                                                 h2[:, cs], start=False,
                                                 stop=True,
                                                 tile_position=(0, 96))
                            nc.vector.tensor_copy(out=Z[g][96:106, :],
                                                  in_=sp[96:106, :])
            interp(nstep)

            # decoder
            for g in range(NG):
                gs = slice(g * GW, (g + 1) * GW)
                ang = pp.tile([128, GW], f32, tag='pp')
                for c in range(GW // 512):
                    cs = slice(c * 512, (c + 1) * 512)
                    nc.tensor.matmul(ang[0:65, cs], fpad, Zdec[g][:, cs],
                                     start=True, stop=True)
                # range-reduce: ang rows hold m = f*x (no 2*pi factor);
                # r = m - round(m) in [-.5,.5] (DVE f32<->i32 casts round to
                # nearest), then sin(2*pi*r) = sin(2*pi*m). cos via m+0.25.
                # row 64 holds pi*x directly (already in range).
                red = hpool.tile([128, GW], f32, tag='h', name='red')
                redi = hpool.tile([128, GW], mybir.dt.int32, tag='h',
                                  name='redi')
                redf = hpool.tile([128, GW], f32, tag='h', name='redf')
                nc.vector.tensor_copy(out=redi[0:16, :], in_=ang[0:16, :])
                nc.vector.tensor_copy(out=redf[0:16, :], in_=redi[0:16, :])
                nc.vector.tensor_tensor(out=red[0:16, :], in0=ang[0:16, :],
                                        in1=redf[0:16, :], op=OP.subtract)
                nc.vector.tensor_scalar(red[32:48, :], ang[32:48, :], 0.25,
                                        None, OP.add)
                nc.vector.tensor_copy(out=redi[32:48, :], in_=red[32:48, :])
                nc.vector.tensor_copy(out=redf[32:48, :], in_=redi[32:48, :])
                nc.vector.tensor_tensor(out=red[32:48, :], in0=red[32:48, :],
                                        in1=redf[32:48, :], op=OP.subtract)
                two_pi = float(2.0 * np.pi)
                nc.scalar.activation(out=Zdec[g][0:16, :], in_=red[0:16, :],
                                     func=AF.Sin, scale=two_pi)
                nc.scalar.activation(out=Zdec[g][32:48, :], in_=red[32:48, :],
                                     func=AF.Sin, scale=two_pi)
                srow = hpool.tile([128, GW], f32, tag='h', name='srow')
                nc.scalar.activation(out=srow[0:1, :], in_=ang[64:65, :],
                                     func=AF.Sin)
                d1 = pp.tile([128, GW], f32, tag='pp')
                for c in range(GW // 512):
                    cs = slice(c * 512, (c + 1) * 512)
                    nc.tensor.matmul(d1[:, cs], dw1, Zdec[g][:, cs],
                                     start=True, stop=True)
                hd1 = hpool.tile([128, GW], f32, tag='h')
                nc.scalar.activation(out=hd1, in_=d1, func=AF.Tanh,
                                     bias=dbias[:, 0:1])
                d2 = pp.tile([128, GW], f32, tag='pp')
                for c in range(GW // 512):
                    cs = slice(c * 512, (c + 1) * 512)
                    nc.tensor.matmul(d2[:, cs], dw2, hd1[:, cs],
                                     start=True, stop=True)
                hd2 = hpool.tile([128, GW], f32, tag='h')
                nc.scalar.activation(out=hd2, in_=d2, func=AF.Tanh,
                                     bias=dbias[:, 1:2])
                d3 = pp.tile([128, GW], f32, tag='pp')
                for c in range(GW // 512):
                    cs = slice(c * 512, (c + 1) * 512)
                    nc.tensor.matmul(d3[:, cs], dw3, hd2[:, cs],
                                     start=True, stop=True)
                hd3 = hpool.tile([128, GW], f32, tag='h')
                nc.scalar.activation(out=hd3, in_=d3, func=AF.Tanh,
                                     bias=dbias[:, 2:3])
                d4 = pp.tile([128, GW], f32, tag='pp')
                for c in range(GW // 512):
                    cs = slice(c * 512, (c + 1) * 512)
                    nc.tensor.matmul(d4[0:1, cs], dw4, hd3[:, cs],
                                     start=True, stop=True)
                # u = (dec + db4) * t - sin(pi x)
                u1 = hpool.tile([128, GW], f32, tag='h', name='u1')
                nc.vector.scalar_tensor_tensor(out=u1[0:1, :],
                                               in0=d4[0:1, :],
                                               scalar=db4c[0:1, 0:1],
                                               in1=trow[g], op0=OP.add,
                                               op1=OP.mult)
                nc.vector.tensor_tensor(out=u1[0:1, :], in0=u1[0:1, :],
                                        in1=srow[0:1, :], op=OP.subtract)
                nc.sync.dma_start(out=u_d.ap()[gs], in_=u1[0:1, :])

    _split_multiwaits(nc, mybir)
    return nc


def _host_prep(inputs, nstep):
    """Compute the derived weight/bias tables shared by all cores."""
    f = {k: np.asarray(v, np.float32) for k, v in inputs.items()}
    pW1, pb1 = f['pW1'], f['pb1']
    pW2, pb2 = f['pW2'], f['pb2']
    pW3, pb3 = f['pW3'], f['pb3']
    dW1, db1 = f['dW1'], f['db1']
    dW2, db2 = f['dW2'], f['db2']
    dW3, db3 = f['dW3'], f['db3']
    dW4, db4 = f['dW4'], f['db4']
    h = np.float64(T_END / nstep)

    W1a = pW1[0:LATENT]          # [10, 128]
    w1t = pW1[LATENT]            # [128]
    w1mu = pW1[LATENT + 1]       # [128]

    # mm1 weights: rows 0-9 k1s, 32-41 k2s, 64-73 k3s, 96-105 a, 106 mu
    w1e = np.zeros((128, 512), np.float64)
    coef = [  # (k1, k2, k3) coefficients per eval
        (0.0, 0.0, 0.0),
        (1.0 / 3.0, 0.0, 0.0),
        (-1.0 / 3.0, 1.0, 0.0),
        (1.0, -1.0, 1.0),
    ]
    for i, (c1, c2, c3) in enumerate(coef):
        blk = w1e[:, i * 128:(i + 1) * 128]
        blk[0:10] = c1 * W1a
        blk[32:42] = c2 * W1a
        blk[64:74] = c3 * W1a
        blk[96:106] = W1a
        blk[106] = w1mu

    w3h = np.zeros((128, LATENT), np.float64)
    w3h[:, :] = h * pW3.astype(np.float64)
    w3h8 = (h / 8.0) * pW3.astype(np.float64)

    selw = np.zeros((128, LATENT), np.float64)
    eye = np.eye(LATENT)
    selw[0:10] = eye / 8.0
    selw[32:42] = 3.0 * eye / 8.0
    selw[64:74] = 3.0 * eye / 8.0
    selw[96:106] = eye

    # per-(step,eval) tanh1 bias: t*w1t + pb1 + (s+gamma_i)*h*(W1a.T @ pb3)
    tgrid = np.linspace(0.0, T_END, nstep + 1).astype(np.float32)
    gammas = np.array([0.0, 1.0 / 3.0, 2.0 / 3.0, 1.0])
    bcorr = (W1a.astype(np.float64).T @ pb3.astype(np.float64)) * h  # [128]
    btab = np.zeros((128, 4 * nstep), np.float64)
    for s in range(nstep):
        for i in range(4):
            te = np.float64(tgrid[s]) + gammas[i] * h
            btab[:, 4 * s + i] = te * w1t + pb1 + (s + gammas[i]) * bcorr

    # decoder weights: Zdec rows 0-15 sin, 32-47 cos, 96-105 alpha,
    # 107 x, 108 t (alpha deficit correction: + (dW1a.T @ pb3) x t)
    dw1 = np.zeros((128, 128), np.float64)
    dw1[0:16] = dW1[0:16]
    dw1[32:48] = dW1[16:32]
    dw1[96:106] = dW1[32:42]
    dw1[108] = dW1[32:42].astype(np.float64).T @ pb3.astype(np.float64)

    freqs = np.linspace(1.0, MAX_FREQ, N_FREQS).astype(np.float32)
    fpad = np.zeros((128, 65), np.float64)
    fpad[107, 0:16] = freqs
    fpad[107, 32:48] = freqs
    fpad[107, 64] = np.pi

    dbias = np.zeros((128, 4), np.float64)
    dbias[:, 0] = db1
    dbias[:, 1] = db2
    dbias[:, 2] = db3
    dbias[:, 3] = -np.pi

    # interpolation hat weights V[s, b] (same idx/ratio math as reference)
    t_all = f['t']
    hf = np.float32(T_END / nstep)
    idx = np.clip(np.floor(t_all / hf).astype(np.int32), 0, nstep - 1)
    ratio = ((t_all - tgrid[idx]) / hf).astype(np.float64)
    V = np.zeros((nstep + 1, B), np.float64)
    rows = np.arange(B)
    np.add.at(V, (idx, rows), 1.0 - ratio)
    np.add.at(V, (idx + 1, rows), ratio)

    shared = {
        'w1e': w1e.astype(np.float32),
        'w2': pW2,
        'w3h': w3h.astype(np.float32),
        'w3h8': w3h8.astype(np.float32),
        'selw': selw.astype(np.float32),
        'btab': btab.astype(np.float32),
        'b2c': pb2.reshape(128, 1),
        'dbias': dbias.astype(np.float32),
        'db4c': np.asarray(db4, np.float32).reshape(1, 1),
        'dw1': dw1.astype(np.float32),
        'dw2': dW2,
        'dw3': dW3,
        'dw4': dW4.reshape(128, 1),
        'fpad': fpad.astype(np.float32),
    }
    in_maps = []
    for c in range(NCORES):
        cs = slice(c * B_CORE, (c + 1) * B_CORE)
        m = dict(shared)
        m['x'] = f['x'][cs]
        m['t'] = f['t'][cs]
        m['mu'] = f['mu'][cs]
        m['V'] = V[:, cs].astype(np.float32)
        in_maps.append(m)
    return in_maps


def _run(inputs, nstep=NSTEP, trace=False):
    from concourse.bass_utils import run_bass_kernel_spmd
    key = nstep
    if key not in _PROG_CACHE:
        _PROG_CACHE[key] = _build(nstep)
    nc = _PROG_CACHE[key]
    in_maps = _host_prep(inputs, nstep)
    res = run_bass_kernel_spmd(nc, in_maps, core_ids=list(range(NCORES)),
                               trace=trace)
    u = np.concatenate([res.results[c]['u'] for c in range(NCORES)])
    return u.astype(np.float32), res


def kernel(**inputs) -> np.ndarray:
    u, _ = _run(inputs)
    return u
